# revision 3
# baseline (speedup 1.0000x reference)
"""Trainium2 Bass kernel for nn_BMSampling: out = X.reshape(B*C, T) @ smp_weight.

Strategy (v3 — unique-column compaction + latency tuning):
- smp_weight columns are <=2-tap linear-interpolation stencils. Beyond the
  ~55.6% all-zero columns, the nonzero columns repeat heavily: only ~6k of
  the 142k nonzero columns are distinct (bitwise). The kernel dedups
  columns at runtime (generic for any weight), computes only the unique
  columns on device, and the host replicates duplicates + scatters zeros
  during assembly. This cuts device HBM traffic ~23x vs computing every
  nonzero column; the baseline was HBM-DMA bound at ~354 GB/s/core.
- Tensor-parallel over unique columns: 8 cores x nsh each. Each core
  computes OUT[512, nsh] = XT[100,512].T @ W[100,nsh].
- Precision: fp16 inputs, single-pass matmul, fp32 PSUM accumulate, fp16
  output written to HBM (halves store bytes) and upcast on host. Each
  output element is a sum of <=2 products, so error is ~3 ulp of fp16
  ~ 1e-3 rel worst case, inside the 2e-2 gate with >10x margin.
- At this size the kernel is latency-bound, so:
  * X loads on the SP HWDGE ring while W strips load on the ACT ring
    (parallel, first matmul ~2us earlier than serialized loads).
  * Junk warmup matmuls run during the load window so the PE HAM clock
    gate (cold = ~1.2 GHz) is released closer to when real work arrives.
  * PSUM->SBUF fp32->fp16 cast copies split across ACT (wide strips) and
    DVE (narrow strips) so neither engine paces the pipeline.
  * The last m-tile's stores are split per strip with their own staging
    tiles, so the final (small) store issues as early as possible and its
    ~2us HBM write-receipt tail starts sooner.
"""

from contextlib import ExitStack

import numpy as np

import concourse.bacc as bacc
import concourse.mybir as mybir
import concourse.tile as tile
from concourse import bass_utils

B, C, T = 4, 128, 100
N_SMP, D_PROP = 32, 100
M = B * C                     # 512 matmul rows
NDT = N_SMP * D_PROP * T      # 320000 output columns
NCORES = 8
COLGRAN = 128 * NCORES        # unique col count padded to this

N_INNER = 512                 # matmul free dim (one PSUM bank of f32)
N_WARM = 5                    # junk matmuls to keep PE busy during loads
F32 = mybir.dt.float32
F16 = mybir.dt.float16

_PROGRAMS = {}


def _build(nsh):
    """Per-core program computing OUT[512, nsh] = XT.T @ W[100, nsh] in fp16."""
    if nsh in _PROGRAMS:
        return _PROGRAMS[nsh]

    widths = [N_INNER] * (nsh // N_INNER)
    if nsh % N_INNER:
        widths.append(nsh % N_INNER)

    nc = bacc.Bacc("TRN2", debug=False)
    xt = nc.dram_tensor("XT", [T, M], F16, kind="ExternalInput").ap()
    wt = nc.dram_tensor("WT", [T, nsh], F16, kind="ExternalInput").ap()
    out = nc.dram_tensor("OUT", [M // 128, 128, nsh], F16, kind="ExternalOutput").ap()

    with tile.TileContext(nc) as tc, ExitStack() as ctx:
        xpool = ctx.enter_context(tc.tile_pool(name="x", bufs=1))
        wpool = ctx.enter_context(tc.tile_pool(name="w", bufs=1))
        opool = ctx.enter_context(tc.tile_pool(name="o", bufs=3))
        o3pool = ctx.enter_context(tc.tile_pool(name="o3", bufs=1))
        warmpool = ctx.enter_context(tc.tile_pool(name="warm", bufs=1))
        pspool = ctx.enter_context(tc.tile_pool(name="ps", bufs=4, space="PSUM"))
        warmps = ctx.enter_context(tc.tile_pool(name="wps", bufs=1, space="PSUM"))

        # PE warmup: the HAM clock gate holds a cold PE at ~half clock and
        # only releases after ~4us of sustained activity. Chew through junk
        # matmuls while the input DMAs are in flight.
        warm_x = warmpool.tile([T, N_INNER], F16)
        nc.vector.memset(warm_x[:], 0.0)
        warm_ps = warmps.tile([128, N_INNER], F32)
        for _ in range(N_WARM):
            nc.tensor.matmul(
                warm_ps[:], warm_x[:, :128], warm_x[:], start=True, stop=True
            )

        # X on the SP ring, W strips on the ACT ring: parallel loads.
        x_sb = xpool.tile([T, M], F16)
        nc.sync.dma_start(out=x_sb[:], in_=xt)

        w_tiles = []
        n0 = 0
        for si, wdt in enumerate(widths):
            w_sb = wpool.tile([T, wdt], F16, tag=f"w{si}")
            nc.scalar.dma_start(out=w_sb[:], in_=wt[:, n0 : n0 + wdt])
            w_tiles.append((n0, wdt, w_sb))
            n0 += wdt

        def cast_copy(dst, src, wdt):
            # ACT is the faster copier; give it the wide strips.
            if wdt > 384:
                nc.scalar.copy(out=dst, in_=src)
            else:
                nc.vector.tensor_copy(out=dst, in_=src)

        nm = M // 128
        for m in range(nm):
            msl = slice(m * 128, (m + 1) * 128)
            if m < nm - 1:
                o_sb = opool.tile([128, nsh], F16, tag="o")
                for n0, wdt, w_sb in w_tiles:
                    ps = pspool.tile([128, N_INNER], F32)
                    nc.tensor.matmul(
                        ps[:, :wdt], x_sb[:, msl], w_sb[:], start=True, stop=True
                    )
                    cast_copy(o_sb[:, n0 : n0 + wdt], ps[:, :wdt], wdt)
                nc.sync.dma_start(out=out[m], in_=o_sb[:])
            else:
                # Last m-tile: per-strip staging + stores so the final store
                # (the smallest strip) issues as soon as its copy lands.
                for si, (n0, wdt, w_sb) in enumerate(w_tiles):
                    ps = pspool.tile([128, N_INNER], F32)
                    nc.tensor.matmul(
                        ps[:, :wdt], x_sb[:, msl], w_sb[:], start=True, stop=True
                    )
                    o3 = o3pool.tile([128, wdt], F16, tag=f"o3_{si}")
                    cast_copy(o3[:], ps[:, :wdt], wdt)
                    nc.sync.dma_start(out=out[m, :, n0 : n0 + wdt], in_=o3[:])

    nc.compile()
    _PROGRAMS[nsh] = nc
    return nc


def _dedup(W):
    """Find unique nonzero columns. Returns (nz, first, inv) with
    W[:, nz[first]] the unique columns and W[:, nz] == W[:, nz[first]][:, inv]."""
    nz = np.flatnonzero((W != 0).any(axis=0))
    Wnz = W[:, nz]
    mask = Wnz != 0
    if len(nz) == 0:
        return nz, np.zeros(0, np.int64), np.zeros(0, np.int64)
    if mask.sum(axis=0).max() <= 2:
        # Fast path: each column is a <=2-tap stencil; key on (row_lo,
        # row_hi, val_lo_bits, val_hi_bits) instead of sorting full columns.
        l = mask.argmax(axis=0).astype(np.uint64)
        r = (W.shape[0] - 1 - mask[::-1].argmax(axis=0)).astype(np.uint64)
        cols = np.arange(Wnz.shape[1])
        wl = np.ascontiguousarray(Wnz[l.astype(np.int64), cols])
        wr = np.ascontiguousarray(Wnz[r.astype(np.int64), cols])
        keys = np.empty((Wnz.shape[1], 2), np.uint64)
        keys[:, 0] = (l << np.uint64(32)) | r
        keys[:, 1] = (
            wl.view(np.uint32).astype(np.uint64) << np.uint64(32)
        ) | wr.view(np.uint32).astype(np.uint64)
        _, first, inv = np.unique(
            keys, axis=0, return_index=True, return_inverse=True
        )
    else:
        _, first, inv = np.unique(
            np.ascontiguousarray(Wnz.T), axis=0, return_index=True, return_inverse=True
        )
    return nz, first.astype(np.int64), inv.reshape(-1).astype(np.int64)


def prepare_run(X, smp_weight):
    """Returns (nc, in_maps, assemble) where assemble(results)->full output."""
    X = np.ascontiguousarray(np.asarray(X, dtype=np.float32))
    Wfull = np.asarray(smp_weight, dtype=np.float32)

    nz, first, inv = _dedup(Wfull)
    U = len(first)
    padded = max(COLGRAN, (U + COLGRAN - 1) // COLGRAN * COLGRAN)
    nsh = padded // NCORES

    Wu = np.zeros((T, padded), dtype=np.float16)
    if U:
        Wu[:, :U] = Wfull[:, nz[first]]
    xt16 = np.ascontiguousarray(X.reshape(M, T).T.astype(np.float16))

    in_maps = [
        {"XT": xt16, "WT": np.ascontiguousarray(Wu[:, i * nsh : (i + 1) * nsh])}
        for i in range(NCORES)
    ]
    nc = _build(nsh)

    def assemble(results):
        compact = np.concatenate(
            [results[i]["OUT"].reshape(M, nsh) for i in range(NCORES)], axis=1
        )
        full = np.zeros((M, NDT), dtype=np.float32)
        if U:
            full[:, nz] = compact[:, :U].astype(np.float32)[:, inv]
        return full.reshape(B, C, N_SMP, D_PROP, T)

    return nc, in_maps, assemble


def kernel(X, smp_weight):
    nc, in_maps, assemble = prepare_run(X, smp_weight)
    res = bass_utils.run_bass_kernel_spmd(nc, in_maps, core_ids=list(range(NCORES)))
    return assemble(res.results)


# revision 6
# speedup vs baseline: 1.1605x; 1.1605x over previous
"""Trainium2 Bass kernel for nn_BMSampling: out = X.reshape(B*C, T) @ smp_weight.

Strategy (v3 — unique-column compaction + latency tuning):
- smp_weight columns are <=2-tap linear-interpolation stencils. Beyond the
  ~55.6% all-zero columns, the nonzero columns repeat heavily: only ~6k of
  the 142k nonzero columns are distinct (bitwise). The kernel dedups
  columns at runtime (generic for any weight), computes only the unique
  columns on device, and the host replicates duplicates + scatters zeros
  during assembly. This cuts device HBM traffic ~23x vs computing every
  nonzero column; the baseline was HBM-DMA bound at ~354 GB/s/core.
- Tensor-parallel over unique columns: 8 cores x nsh each. Each core
  computes OUT[512, nsh] = XT[100,512].T @ W[100,nsh].
- Precision: fp16 inputs, single-pass matmul, fp32 PSUM accumulate, fp16
  output written to HBM (halves store bytes) and upcast on host. Each
  output element is a sum of <=2 products, so error is ~3 ulp of fp16
  ~ 1e-3 rel worst case, inside the 2e-2 gate with >10x margin.
- At this size the kernel is latency-bound, so:
  * X and the first W strip load as ONE combined DMA (2KB partition
    lines) on the otherwise-idle SP ring; the remaining W strip follows.
    The ACT ring only carries its hoisted ACT_TABLE_LOAD, off the load
    path. Fewer DMAs = fewer ~2us completion-receipt latencies in series.
  * enable_partition_id=False drops the per-engine partition-id
    TENSOR_LOADs (~1.4us of NEFF preamble); this kernel never branches
    on core id.
  * PSUM->SBUF fp32->fp16 cast copies split across DVE (wide strips,
    1.35 ns/col) and ACT (narrow strips) so neither engine paces.
  * Stores are one DMA per 128-row m-tile on the SP ring, streaming out
    behind compute; only the last store's ~2us HBM receipt is exposed.
"""

from contextlib import ExitStack

import numpy as np

import concourse.bacc as bacc
import concourse.mybir as mybir
import concourse.tile as tile
from concourse import bass_utils

B, C, T = 4, 128, 100
N_SMP, D_PROP = 32, 100
M = B * C                     # 512 matmul rows
NDT = N_SMP * D_PROP * T      # 320000 output columns
NCORES = 8
COLGRAN = 128 * NCORES        # unique col count padded to this

N_INNER = 512                 # matmul free dim (one PSUM bank of f32)
F32 = mybir.dt.float32
F16 = mybir.dt.float16

_PROGRAMS = {}


def _build(nsh):
    """Per-core program computing OUT[512, nsh] = XT.T @ W[100, nsh] in fp16.

    Inputs are packed as XW = [X | W_strip0] (one DMA) plus WR = the
    remaining W columns (second DMA), both on the SP ring.
    """
    if nsh in _PROGRAMS:
        return _PROGRAMS[nsh]

    w0 = min(N_INNER, nsh)
    widths = [w0]
    rest = nsh - w0
    widths += [N_INNER] * (rest // N_INNER)
    if rest % N_INNER:
        widths.append(rest % N_INNER)

    nc = bacc.Bacc("TRN2", debug=False, enable_partition_id=False)
    xw = nc.dram_tensor("XW", [T, M + w0], F16, kind="ExternalInput").ap()
    if rest:
        wr = nc.dram_tensor("WR", [T, rest], F16, kind="ExternalInput").ap()
    out = nc.dram_tensor("OUT", [M // 128, 128, nsh], F16, kind="ExternalOutput").ap()

    with tile.TileContext(nc) as tc, ExitStack() as ctx:
        xwpool = ctx.enter_context(tc.tile_pool(name="xw", bufs=1))
        wrpool = ctx.enter_context(tc.tile_pool(name="wr", bufs=1))
        opool = ctx.enter_context(tc.tile_pool(name="o", bufs=4))
        pspool = ctx.enter_context(tc.tile_pool(name="ps", bufs=4, space="PSUM"))

        xw_sb = xwpool.tile([T, M + w0], F16)
        nc.sync.dma_start(out=xw_sb[:], in_=xw)
        x_sb = xw_sb[:, :M]

        w_tiles = [(0, w0, xw_sb[:, M : M + w0])]
        if rest:
            wr_sb = wrpool.tile([T, rest], F16)
            nc.sync.dma_start(out=wr_sb[:], in_=wr)
            n0 = w0
            for wdt in widths[1:]:
                w_tiles.append((n0, wdt, wr_sb[:, n0 - w0 : n0 - w0 + wdt]))
                n0 += wdt

        def cast_copy(dst, src, wdt):
            # DVE is the faster cast-copier (~1.35 ns/col vs ACT ~1.6).
            if wdt > 384:
                nc.vector.tensor_copy(out=dst, in_=src)
            else:
                nc.scalar.copy(out=dst, in_=src)

        for m in range(M // 128):
            msl = slice(m * 128, (m + 1) * 128)
            o_sb = opool.tile([128, nsh], F16, tag="o")
            for n0, wdt, w_ap in w_tiles:
                ps = pspool.tile([128, N_INNER], F32)
                nc.tensor.matmul(
                    ps[:, :wdt], x_sb[:, msl], w_ap, start=True, stop=True
                )
                cast_copy(o_sb[:, n0 : n0 + wdt], ps[:, :wdt], wdt)
            nc.sync.dma_start(out=out[m], in_=o_sb[:])

    nc.compile()
    _PROGRAMS[nsh] = nc
    return nc


def _dedup(W):
    """Find unique nonzero columns. Returns (nz, first, inv) with
    W[:, nz[first]] the unique columns and W[:, nz] == W[:, nz[first]][:, inv]."""
    nz = np.flatnonzero((W != 0).any(axis=0))
    Wnz = W[:, nz]
    mask = Wnz != 0
    if len(nz) == 0:
        return nz, np.zeros(0, np.int64), np.zeros(0, np.int64)
    if mask.sum(axis=0).max() <= 2:
        # Fast path: each column is a <=2-tap stencil; key on (row_lo,
        # row_hi, val_lo_bits, val_hi_bits) instead of sorting full columns.
        l = mask.argmax(axis=0).astype(np.uint64)
        r = (W.shape[0] - 1 - mask[::-1].argmax(axis=0)).astype(np.uint64)
        cols = np.arange(Wnz.shape[1])
        wl = np.ascontiguousarray(Wnz[l.astype(np.int64), cols])
        wr = np.ascontiguousarray(Wnz[r.astype(np.int64), cols])
        keys = np.empty((Wnz.shape[1], 2), np.uint64)
        keys[:, 0] = (l << np.uint64(32)) | r
        keys[:, 1] = (
            wl.view(np.uint32).astype(np.uint64) << np.uint64(32)
        ) | wr.view(np.uint32).astype(np.uint64)
        _, first, inv = np.unique(
            keys, axis=0, return_index=True, return_inverse=True
        )
    else:
        _, first, inv = np.unique(
            np.ascontiguousarray(Wnz.T), axis=0, return_index=True, return_inverse=True
        )
    return nz, first.astype(np.int64), inv.reshape(-1).astype(np.int64)


def prepare_run(X, smp_weight):
    """Returns (nc, in_maps, assemble) where assemble(results)->full output."""
    X = np.ascontiguousarray(np.asarray(X, dtype=np.float32))
    Wfull = np.asarray(smp_weight, dtype=np.float32)

    nz, first, inv = _dedup(Wfull)
    U = len(first)
    padded = max(COLGRAN, (U + COLGRAN - 1) // COLGRAN * COLGRAN)
    nsh = padded // NCORES

    Wu = np.zeros((T, padded), dtype=np.float16)
    if U:
        Wu[:, :U] = Wfull[:, nz[first]]
    xt16 = X.reshape(M, T).T.astype(np.float16)

    w0 = min(N_INNER, nsh)
    in_maps = []
    for i in range(NCORES):
        shard = Wu[:, i * nsh : (i + 1) * nsh]
        m = {"XW": np.ascontiguousarray(np.concatenate([xt16, shard[:, :w0]], axis=1))}
        if nsh > w0:
            m["WR"] = np.ascontiguousarray(shard[:, w0:])
        in_maps.append(m)
    nc = _build(nsh)

    def assemble(results):
        compact = np.concatenate(
            [results[i]["OUT"].reshape(M, nsh) for i in range(NCORES)], axis=1
        )
        full = np.zeros((M, NDT), dtype=np.float32)
        if U:
            full[:, nz] = compact[:, :U].astype(np.float32)[:, inv]
        return full.reshape(B, C, N_SMP, D_PROP, T)

    return nc, in_maps, assemble


def kernel(X, smp_weight):
    nc, in_maps, assemble = prepare_run(X, smp_weight)
    res = bass_utils.run_bass_kernel_spmd(nc, in_maps, core_ids=list(range(NCORES)))
    return assemble(res.results)
